# revision 14
# baseline (speedup 1.0000x reference)
"""Trainium2 Bass kernel for an autoregressive decoder layer (decode step).

Shapes (full): B=1024, E=128, H=8 heads x HD=16, cross-attn ctx N1=1001,
self-attn KV cache T_PREV=511 (+1 computed token -> 512).

Sharding: pure data parallel over 8 NeuronCores; 128 batches per core,
weights replicated. No collectives. On-chip layout: partition = local batch.

Head semantics (faithful to the reference's raw reshape [B,S,E]->[B*H,S,HD]):
head h of a key/value buffer reads the flat (S*E) per-batch buffer at
offsets h*S*HD + t*HD + d. The query (S=1) uses the clean E-slice per head.

v2 design:
- K/V loaded with SWDGE (gpsimd) DMAs that cast f32->f16 inline, in
  half-head-or-larger chunks (~4MB HBM reads) for high DMA efficiency.
- Compute tiles of TH=256 positions; q pre-scaled by 1/sqrt(HD) and cast
  to f16 once per head.
- exp runs on the ACT engine reading the score row broadcast along HD, so
  it directly materializes p[t] replicated 16x as a dense f16 tensor
  (pbig); the p*v multiply is then a unit-stride f16 tensor_tensor (2x
  DVE mode). accum_out on the same op yields 16*sum(exp) per tile.
- All elementwise/tree work in f16 on DVE at 2x mode.
"""

import sys
from contextlib import ExitStack

import numpy as np

if "/opt/trn_rl_repo" not in sys.path:
    sys.path.insert(0, "/opt/trn_rl_repo")

import concourse.bacc as bacc
import concourse.bass as bass
import concourse.mybir as mybir
from concourse.tile import TileContext
from concourse.bass_utils import run_bass_kernel_spmd
from concourse.masks import make_identity

F32 = mybir.dt.float32
F16 = mybir.dt.float16
U8 = mybir.dt.uint8

B = 1024
E = 128
H = 8
HD = 16
N1 = 1001
T_PREV = 511
NCORES = 8
BL = B // NCORES  # 128 batches per core
EPS = 1e-5
TH = 512          # seq positions per compute tile
CH = 512          # seq positions per DMA chunk (upper bound)
NEG = -30000.0    # f16-safe "minus infinity" for the mask

WNAMES = ["Wk", "Wv", "W0sa", "Wqatt", "W0att", "W1", "W2"]


def build_kernel(bl=BL, n1=N1, t_prev=T_PREV, repeat=1, mode="full", ch=CH,
                 th=TH):
    nc = bacc.Bacc("TRN2", target_bir_lowering=False, debug=False,
                   num_devices=NCORES)

    # ---- dram parameters ----
    d_ht = nc.declare_dram_parameter("h_t", [bl, E], F32, isOutput=False)
    d_katt = nc.declare_dram_parameter("K_att", [bl, n1, E], F32, isOutput=False)
    d_vatt = nc.declare_dram_parameter("V_att", [bl, n1, E], F32, isOutput=False)
    d_ksa = nc.declare_dram_parameter("K_sa_prev", [bl, t_prev, E], F32, isOutput=False)
    d_vsa = nc.declare_dram_parameter("V_sa_prev", [bl, t_prev, E], F32, isOutput=False)
    d_mask = nc.declare_dram_parameter("mask", [bl, n1], U8, isOutput=False)
    d_w = {}
    d_b = {}
    for w in WNAMES:
        d_w[w] = nc.declare_dram_parameter(w + "_w", [E, E], F32, isOutput=False)
        d_b[w] = nc.declare_dram_parameter(w + "_b", [1, E], F32, isOutput=False)
    d_lng = {}
    d_lnb = {}
    for ln in ["ln_sa", "ln_ff"]:
        d_lng[ln] = nc.declare_dram_parameter(ln + "_g", [1, E], F32, isOutput=False)
        d_lnb[ln] = nc.declare_dram_parameter(ln + "_b", [1, E], F32, isOutput=False)
    d_out = nc.declare_dram_parameter("out", [bl, E], F32, isOutput=True)

    with TileContext(nc) as tc, ExitStack() as ctx:
        const = ctx.enter_context(tc.tile_pool(name="const", bufs=1))
        xpool = ctx.enter_context(tc.tile_pool(name="xpool", bufs=2))
        kpool = ctx.enter_context(tc.tile_pool(name="kpool", bufs=2))
        vpool = ctx.enter_context(tc.tile_pool(name="vpool", bufs=2))
        ppool = ctx.enter_context(tc.tile_pool(name="ppool", bufs=2))
        spool = ctx.enter_context(tc.tile_pool(name="spool", bufs=2))
        acc = ctx.enter_context(tc.tile_pool(name="acc", bufs=2))
        small = ctx.enter_context(tc.tile_pool(name="small", bufs=4))
        psum = ctx.enter_context(tc.tile_pool(name="psum", bufs=2, space="PSUM"))

        # ---- constants ----
        ident = const.tile([128, 128], F32)
        make_identity(nc, ident[:])
        eps_t = const.tile([128, 1], F32)
        nc.vector.memset(eps_t[:], EPS)

        # weights transposed to [e_in, e_out]; biases broadcast to [128, E]
        wt = {}
        bfull = {}
        for w in WNAMES:
            wsb = xpool.tile([E, E], F32, tag="wstage")
            nc.sync.dma_start(out=wsb[:], in_=d_w[w][:])
            pst = psum.tile([E, E], F32, tag="pst")
            nc.tensor.transpose(pst[:], wsb[:], ident[:])
            wt[w] = const.tile([E, E], F32, tag="wt_" + w, name="wt_" + w)
            nc.any.tensor_copy(wt[w][:], pst[:])
            bfull[w] = const.tile([128, E], F32, tag="bf_" + w, name="bf_" + w)
            nc.gpsimd.dma_start(out=bfull[w][:],
                                in_=d_b[w].ap().partition_broadcast(128))
        lngf = {}
        lnbf = {}
        for ln in ["ln_sa", "ln_ff"]:
            lngf[ln] = const.tile([128, E], F32, tag="lng_" + ln, name="lng_" + ln)
            nc.gpsimd.dma_start(out=lngf[ln][:],
                                in_=d_lng[ln].ap().partition_broadcast(128))
            lnbf[ln] = const.tile([128, E], F32, tag="lnb_" + ln, name="lnb_" + ln)
            nc.gpsimd.dma_start(out=lnbf[ln][:],
                                in_=d_lnb[ln].ap().partition_broadcast(128))

        # mask -> f16 negmask (NEG where masked, 0 elsewhere); padded to the
        # tile multiple with NEG so partial tiles can run the full-width path
        n1_pad = ((n1 + th - 1) // th) * th
        m8 = const.tile([128, n1], U8)
        nc.sync.dma_start(out=m8[:], in_=d_mask[:])
        negmask = const.tile([128, n1_pad], F16)
        nc.vector.tensor_scalar(negmask[:, :n1], m8[:], NEG, None,
                                mybir.AluOpType.mult)
        if n1_pad > n1:
            nc.vector.memset(negmask[:, n1:], NEG)

        # h_t
        ht = const.tile([128, E], F32)
        nc.sync.dma_start(out=ht[:], in_=d_ht[:])

        # ---- helpers ----
        def linear(x, w, out, extra_add=None):
            """out = x @ W^T + b (+ extra_add). x, out: [128, E] sbuf f32."""
            pst = psum.tile([E, E], F32, tag="pst")
            nc.tensor.transpose(pst[:], x[:], ident[:])
            xt = xpool.tile([E, E], F32, tag="xt")
            nc.any.tensor_copy(xt[:], pst[:])
            yps = psum.tile([128, E], F32, tag="yps")
            nc.tensor.matmul(yps[:], xt[:], wt[w][:], start=True, stop=True)
            if extra_add is None:
                nc.vector.tensor_add(out[:], yps[:], bfull[w][:])
            else:
                tmp = xpool.tile([128, E], F32, tag="lin_tmp")
                nc.vector.tensor_add(tmp[:], yps[:], bfull[w][:])
                nc.vector.tensor_add(out[:], tmp[:], extra_add[:])

        def layernorm(x, ln, out):
            stats = small.tile([128, 6], F32, tag="bn_stats")
            nc.vector.bn_stats(stats[:], x[:])
            mv = small.tile([128, 2], F32, tag="bn_mv")
            nc.vector.bn_aggr(mv[:], stats[:])
            std = small.tile([128, 1], F32, tag="std")
            nc.scalar.activation(std[:], mv[:, 1:2],
                                 mybir.ActivationFunctionType.Sqrt,
                                 bias=eps_t[:], scale=1.0)
            rstd = small.tile([128, 1], F32, tag="rstd")
            nc.vector.reciprocal(rstd[:], std[:])
            xn = xpool.tile([128, E], F32, tag="ln_xn")
            nc.vector.tensor_scalar(xn[:], x[:], mv[:, 0:1], rstd[:],
                                    mybir.AluOpType.subtract,
                                    mybir.AluOpType.mult)
            xg = xpool.tile([128, E], F32, tag="ln_xg")
            nc.vector.tensor_mul(xg[:], xn[:], lngf[ln][:])
            nc.vector.tensor_add(out[:], xg[:], lnbf[ln][:])

        def attention(q, kd, vd, s_tot, n_prev, kv_extra, masked, a_out):
            """Batched MHA decode, raw-reshape head semantics.
            q: [128, E] sbuf f32. kd/vd: dram [bl, n_prev, E] flat-viewed.
            s_tot: total positions per head (n_prev, or n_prev+1 w/ extra).
            kv_extra: None or (k_new, v_new) [128, E] f32 appended flat-end.
            a_out: [128, E] sbuf f32."""
            nflat_prev = n_prev * E
            ntiles = (s_tot + th - 1) // th
            nchunk = (s_tot + ch - 1) // ch
            kflat = kd[:].rearrange("b t e -> b (t e)")
            vflat = vd[:].rearrange("b t e -> b (t e)")
            dparts = acc.tile([128, H, ntiles], F32, tag="dparts")
            oparts = acc.tile([128, H, ntiles, HD], F32, tag="oparts")
            for h in range(H):
                # q for this head, pre-scaled by 1/sqrt(HD), in f16
                qh16 = small.tile([128, HD], F16, tag="qh16", name="qh16")
                nc.vector.tensor_scalar(qh16[:], q[:, h * HD:(h + 1) * HD],
                                        0.25, None, mybir.AluOpType.mult)
                qb = qh16[:].unsqueeze(1).broadcast_to([128, th, HD])
                for c in range(nchunk):
                    c0 = c * ch                      # chunk start position
                    cp = min(ch, s_tot - c0)         # positions in chunk
                    f0 = h * s_tot * HD + c0 * HD    # flat float offset
                    avail = max(0, min(cp * HD, nflat_prev - f0))
                    ndp = avail // HD                # positions from dram
                    kvdt = F16 if mode in ("full", "dmaonly") else F32
                    dma_eng = nc.sync if mode == "dmaonly_hw" else nc.gpsimd
                    kc = kpool.tile([128, ch, HD], kvdt, tag="kc")
                    vc = vpool.tile([128, ch, HD], kvdt, tag="vc")
                    if ndp > 0:
                        dma_eng.dma_start(
                            out=kc[:, :ndp, :],
                            in_=kflat[:, f0:f0 + ndp * HD].rearrange(
                                "b (t d) -> b t d", d=HD))
                        dma_eng.dma_start(
                            out=vc[:, :ndp, :],
                            in_=vflat[:, f0:f0 + ndp * HD].rearrange(
                                "b (t d) -> b t d", d=HD))
                    if ndp < cp:
                        # tail comes from the freshly-computed k/v token
                        e0 = f0 + ndp * HD - nflat_prev
                        ncp = (cp - ndp) * HD
                        nc.vector.tensor_copy(
                            kc[:, ndp:cp, :],
                            kv_extra[0][:, e0:e0 + ncp].rearrange(
                                "b (t d) -> b t d", d=HD))
                        nc.vector.tensor_copy(
                            vc[:, ndp:cp, :],
                            kv_extra[1][:, e0:e0 + ncp].rearrange(
                                "b (t d) -> b t d", d=HD))
                    ctiles = (cp + th - 1) // th
                    if cp < ctiles * th:
                        # zero-pad so padded scores exp to 0 via the NEG
                        # negmask pad (and pv pad is 0 * 0)
                        nc.vector.memset(kc[:, cp:ctiles * th, :], 0.0)
                        nc.vector.memset(vc[:, cp:ctiles * th, :], 0.0)
                    if mode.startswith("dmaonly"):
                        nc.vector.tensor_copy(
                            dparts[:, h, (c0 // th):(c0 // th) + 1],
                            kc[:, 0, 0:1])
                        nc.vector.tensor_copy(oparts[:, h, c0 // th, :],
                                              vc[:, 0, :])
                        continue
                    for i in range(ctiles):
                        t0 = i * th                  # offset within chunk
                        g0 = c0 + t0                 # global position
                        it = g0 // th                # global tile index
                        kt = kc[:, t0:t0 + th, :]
                        vt = vc[:, t0:t0 + th, :]
                        prod = ppool.tile([128, th, HD], F16, tag="prod",
                                          bufs=1)
                        nc.vector.tensor_mul(prod[:], kt, qb)
                        t8 = spool.tile([128, th, 8], F16, tag="t8",
                                        bufs=1)
                        nc.vector.tensor_add(t8[:], prod[:, :, 0:8],
                                             prod[:, :, 8:16])
                        t4 = spool.tile([128, th, 4], F16, tag="t4",
                                        bufs=1)
                        nc.vector.tensor_add(t4[:], t8[:, :, 0:4],
                                             t8[:, :, 4:8])
                        t2 = spool.tile([128, th, 2], F16, tag="t2",
                                        bufs=1)
                        nc.vector.tensor_add(t2[:], t4[:, :, 0:2],
                                             t4[:, :, 2:4])
                        s_raw = spool.tile([128, th], F16, tag="s_raw")
                        nc.vector.tensor_add(s_raw[:], t2[:, :, 0],
                                             t2[:, :, 1])
                        if masked:
                            s_in = spool.tile([128, th], F16, tag="s_msk")
                            nc.vector.tensor_add(s_in[:], s_raw[:],
                                                 negmask[:, g0:g0 + th])
                        else:
                            s_in = s_raw
                        # ACT: pbig[p,t,d] = exp(s[p,t]) replicated over d;
                        # accum_out = HD * sum_t exp (corrected at combine)
                        pbig = ppool.tile([128, th, HD], F16, tag="pbig")
                        nc.scalar.activation(
                            pbig[:],
                            s_in[:].unsqueeze(2).broadcast_to([128, th, HD]),
                            mybir.ActivationFunctionType.Exp,
                            accum_out=dparts[:, h, it:it + 1])
                        pv = ppool.tile([128, th, HD], F16, tag="pv",
                                        bufs=1)
                        nc.vector.tensor_mul(pv[:], vt, pbig[:])
                        pv2 = ppool.tile([128, th // 2, HD], F16, tag="pv2",
                                         bufs=1)
                        nc.vector.tensor_add(pv2[:], pv[:, 0:th // 2, :],
                                             pv[:, th // 2:th, :])
                        pv3 = ppool.tile([128, th // 4, HD], F16, tag="pv3",
                                         bufs=1)
                        nc.vector.tensor_add(pv3[:], pv2[:, 0:th // 4, :],
                                             pv2[:, th // 4:th // 2, :])
                        pv4 = ppool.tile([128, th // 8, HD], F16, tag="pv4",
                                         bufs=1)
                        nc.vector.tensor_add(pv4[:], pv3[:, 0:th // 8, :],
                                             pv3[:, th // 8:th // 4, :])
                        pv5 = ppool.tile([128, th // 16, HD], F16, tag="pv5",
                                         bufs=1)
                        nc.vector.tensor_add(pv5[:], pv4[:, 0:th // 16, :],
                                             pv4[:, th // 16:th // 8, :])
                        nc.vector.tensor_reduce(
                            oparts[:, h, it, :], pv5[:].transpose([0, 2, 1]),
                            mybir.AxisListType.X, mybir.AluOpType.add)
            d = small.tile([128, H], F32, tag="attn_d")
            nc.vector.tensor_reduce(d[:], dparts[:], mybir.AxisListType.X,
                                    mybir.AluOpType.add)
            r = small.tile([128, H], F32, tag="attn_r")
            nc.vector.reciprocal(r[:], d[:])
            o = xpool.tile([128, E], F32, tag="attn_o")
            nc.vector.tensor_reduce(o[:].rearrange("p (h d) -> p h d", h=H),
                                    oparts[:].transpose([0, 1, 3, 2]),
                                    mybir.AxisListType.X, mybir.AluOpType.add)
            rb = r[:].unsqueeze(2).broadcast_to([128, H, HD])
            # a = o * (HD / d): the HD factor undoes the broadcast accum
            nc.vector.scalar_tensor_tensor(
                a_out[:].rearrange("p (h d) -> p h d", h=H),
                o[:].rearrange("p (h d) -> p h d", h=H), float(HD), rb,
                mybir.AluOpType.mult, mybir.AluOpType.mult)

        # ---- model ----
        for _rep in range(repeat):
            k_sa = xpool.tile([128, E], F32, tag="k_sa", name="k_sa")
            linear(ht, "Wk", k_sa)
            v_sa = xpool.tile([128, E], F32, tag="v_sa", name="v_sa")
            linear(ht, "Wv", v_sa)

            a_sa = xpool.tile([128, E], F32, tag="a_sa", name="a_sa")
            attention(ht, d_ksa, d_vsa, t_prev + 1, t_prev, (k_sa, v_sa),
                      False, a_sa)

            h1 = xpool.tile([128, E], F32, tag="h1", name="h1")
            linear(a_sa, "W0sa", h1, extra_add=ht)
            h1ln = xpool.tile([128, E], F32, tag="h1ln", name="h1ln")
            layernorm(h1, "ln_sa", h1ln)

            q = xpool.tile([128, E], F32, tag="q", name="q")
            linear(h1ln, "Wqatt", q)
            a_att = xpool.tile([128, E], F32, tag="a_att", name="a_att")
            attention(q, d_katt, d_vatt, n1, n1, None, True, a_att)

            h2 = xpool.tile([128, E], F32, tag="h2", name="h2")
            linear(a_att, "W0att", h2, extra_add=h1ln)
            h2ln = xpool.tile([128, E], F32, tag="h2ln", name="h2ln")
            layernorm(h2, "ln_sa", h2ln)

            ff_pre = xpool.tile([128, E], F32, tag="ff_pre", name="ff_pre")
            linear(h2ln, "W1", ff_pre)
            ff = xpool.tile([128, E], F32, tag="ff", name="ff")
            nc.scalar.activation(ff[:], ff_pre[:],
                                 mybir.ActivationFunctionType.Relu)
            h3 = xpool.tile([128, E], F32, tag="h3", name="h3")
            linear(ff, "W2", h3, extra_add=h2ln)
            h3ln = xpool.tile([128, E], F32, tag="h3ln", name="h3ln")
            layernorm(h3, "ln_ff", h3ln)

            nc.sync.dma_start(out=d_out[:], in_=h3ln[:])

    nc.compile()
    return nc


_NC_CACHE = {}


def _get_nc():
    key = (BL, N1, T_PREV)
    if key not in _NC_CACHE:
        _NC_CACHE[key] = build_kernel()
    return _NC_CACHE[key]


def make_in_maps(inputs, bl=BL, ncores=NCORES):
    """Shard batch dim across cores; replicate weights."""
    in_maps = []
    for c in range(ncores):
        sl = slice(c * bl, (c + 1) * bl)
        m = {}
        # asarray(dtype=...) only copies when conversion is needed; batch
        # slices of C-contiguous inputs pass through zero-copy.
        m["h_t"] = np.ascontiguousarray(np.asarray(
            inputs["h_t"], dtype=np.float32)[sl].reshape(bl, E))
        for k in ["K_att", "V_att", "K_sa_prev", "V_sa_prev"]:
            m[k] = np.ascontiguousarray(np.asarray(inputs[k],
                                                   dtype=np.float32)[sl])
        m["mask"] = np.ascontiguousarray(
            np.asarray(inputs["mask"], dtype=np.uint8)[sl])
        for w in WNAMES:
            m[w + "_w"] = np.ascontiguousarray(inputs[w + "_w"].astype(np.float32))
            m[w + "_b"] = np.ascontiguousarray(
                inputs[w + "_b"].reshape(1, E).astype(np.float32))
        for ln in ["ln_sa", "ln_ff"]:
            m[ln + "_g"] = np.ascontiguousarray(
                inputs[ln + "_g"].reshape(1, E).astype(np.float32))
            m[ln + "_b"] = np.ascontiguousarray(
                inputs[ln + "_b"].reshape(1, E).astype(np.float32))
        in_maps.append(m)
    return in_maps


def kernel(**inputs):
    nc = _get_nc()
    in_maps = make_in_maps(inputs)
    res = run_bass_kernel_spmd(nc, in_maps, core_ids=list(range(NCORES)))
    outs = [res.results[i]["out"].reshape(BL, 1, E) for i in range(NCORES)]
    return np.concatenate(outs, axis=0)


# revision 15
# speedup vs baseline: 1.1276x; 1.1276x over previous
"""Trainium2 Bass kernel for an autoregressive decoder layer (decode step).

Shapes (full): B=1024, E=128, H=8 heads x HD=16, cross-attn ctx N1=1001,
self-attn KV cache T_PREV=511 (+1 computed token -> 512).

Sharding: pure data parallel over 8 NeuronCores; 128 batches per core,
weights replicated. No collectives. On-chip layout: partition = local batch.

Head semantics (faithful to the reference's raw reshape [B,S,E]->[B*H,S,HD]):
head h of a key/value buffer reads the flat (S*E) per-batch buffer at
offsets h*S*HD + t*HD + d. The query (S=1) uses the clean E-slice per head.

v2 design:
- K/V loaded with SWDGE (gpsimd) DMAs that cast f32->f16 inline, in
  half-head-or-larger chunks (~4MB HBM reads) for high DMA efficiency.
- Compute tiles of TH=256 positions; q pre-scaled by 1/sqrt(HD) and cast
  to f16 once per head.
- exp runs on the ACT engine reading the score row broadcast along HD, so
  it directly materializes p[t] replicated 16x as a dense f16 tensor
  (pbig); the p*v multiply is then a unit-stride f16 tensor_tensor (2x
  DVE mode). accum_out on the same op yields 16*sum(exp) per tile.
- All elementwise/tree work in f16 on DVE at 2x mode.
"""

import sys
from contextlib import ExitStack

import numpy as np

if "/opt/trn_rl_repo" not in sys.path:
    sys.path.insert(0, "/opt/trn_rl_repo")

import concourse.bacc as bacc
import concourse.bass as bass
import concourse.mybir as mybir
from concourse.tile import TileContext
from concourse.bass_utils import run_bass_kernel_spmd
from concourse.masks import make_identity

F32 = mybir.dt.float32
F16 = mybir.dt.float16
U8 = mybir.dt.uint8

B = 1024
E = 128
H = 8
HD = 16
N1 = 1001
T_PREV = 511
NCORES = 8
BL = B // NCORES  # 128 batches per core
EPS = 1e-5
TH = 256          # seq positions per compute tile
CH = 512          # seq positions per DMA chunk (upper bound)
NEG = -30000.0    # f16-safe "minus infinity" for the mask

WNAMES = ["Wk", "Wv", "W0sa", "Wqatt", "W0att", "W1", "W2"]


def build_kernel(bl=BL, n1=N1, t_prev=T_PREV, repeat=1, mode="full", ch=CH,
                 th=TH):
    nc = bacc.Bacc("TRN2", target_bir_lowering=False, debug=False,
                   num_devices=NCORES)

    # ---- dram parameters ----
    d_ht = nc.declare_dram_parameter("h_t", [bl, E], F32, isOutput=False)
    d_katt = nc.declare_dram_parameter("K_att", [bl, n1, E], F32, isOutput=False)
    d_vatt = nc.declare_dram_parameter("V_att", [bl, n1, E], F32, isOutput=False)
    d_ksa = nc.declare_dram_parameter("K_sa_prev", [bl, t_prev, E], F32, isOutput=False)
    d_vsa = nc.declare_dram_parameter("V_sa_prev", [bl, t_prev, E], F32, isOutput=False)
    d_mask = nc.declare_dram_parameter("mask", [bl, n1], U8, isOutput=False)
    d_w = {}
    d_b = {}
    for w in WNAMES:
        d_w[w] = nc.declare_dram_parameter(w + "_w", [E, E], F32, isOutput=False)
        d_b[w] = nc.declare_dram_parameter(w + "_b", [1, E], F32, isOutput=False)
    d_lng = {}
    d_lnb = {}
    for ln in ["ln_sa", "ln_ff"]:
        d_lng[ln] = nc.declare_dram_parameter(ln + "_g", [1, E], F32, isOutput=False)
        d_lnb[ln] = nc.declare_dram_parameter(ln + "_b", [1, E], F32, isOutput=False)
    d_out = nc.declare_dram_parameter("out", [bl, E], F32, isOutput=True)

    with TileContext(nc) as tc, ExitStack() as ctx:
        const = ctx.enter_context(tc.tile_pool(name="const", bufs=1))
        xpool = ctx.enter_context(tc.tile_pool(name="xpool", bufs=2))
        kpool = ctx.enter_context(tc.tile_pool(name="kpool", bufs=2))
        vpool = ctx.enter_context(tc.tile_pool(name="vpool", bufs=2))
        ppool = ctx.enter_context(tc.tile_pool(name="ppool", bufs=2))
        spool = ctx.enter_context(tc.tile_pool(name="spool", bufs=2))
        acc = ctx.enter_context(tc.tile_pool(name="acc", bufs=2))
        small = ctx.enter_context(tc.tile_pool(name="small", bufs=4))
        psum = ctx.enter_context(tc.tile_pool(name="psum", bufs=2, space="PSUM"))

        # ---- constants ----
        ident = const.tile([128, 128], F32)
        make_identity(nc, ident[:])
        eps_t = const.tile([128, 1], F32)
        nc.vector.memset(eps_t[:], EPS)

        # weights transposed to [e_in, e_out]; biases broadcast to [128, E]
        wt = {}
        bfull = {}
        for w in WNAMES:
            wsb = xpool.tile([E, E], F32, tag="wstage")
            nc.sync.dma_start(out=wsb[:], in_=d_w[w][:])
            pst = psum.tile([E, E], F32, tag="pst")
            nc.tensor.transpose(pst[:], wsb[:], ident[:])
            wt[w] = const.tile([E, E], F32, tag="wt_" + w, name="wt_" + w)
            nc.any.tensor_copy(wt[w][:], pst[:])
            bfull[w] = const.tile([128, E], F32, tag="bf_" + w, name="bf_" + w)
            nc.gpsimd.dma_start(out=bfull[w][:],
                                in_=d_b[w].ap().partition_broadcast(128))
        lngf = {}
        lnbf = {}
        for ln in ["ln_sa", "ln_ff"]:
            lngf[ln] = const.tile([128, E], F32, tag="lng_" + ln, name="lng_" + ln)
            nc.gpsimd.dma_start(out=lngf[ln][:],
                                in_=d_lng[ln].ap().partition_broadcast(128))
            lnbf[ln] = const.tile([128, E], F32, tag="lnb_" + ln, name="lnb_" + ln)
            nc.gpsimd.dma_start(out=lnbf[ln][:],
                                in_=d_lnb[ln].ap().partition_broadcast(128))

        # mask -> f16 negmask (NEG where masked, 0 elsewhere); padded to the
        # tile multiple with NEG so partial tiles can run the full-width path
        n1_pad = ((n1 + th - 1) // th) * th
        m8 = const.tile([128, n1], U8)
        nc.sync.dma_start(out=m8[:], in_=d_mask[:])
        negmask = const.tile([128, n1_pad], F16)
        nc.vector.tensor_scalar(negmask[:, :n1], m8[:], NEG, None,
                                mybir.AluOpType.mult)
        if n1_pad > n1:
            nc.vector.memset(negmask[:, n1:], NEG)

        # h_t
        ht = const.tile([128, E], F32)
        nc.sync.dma_start(out=ht[:], in_=d_ht[:])

        # ---- helpers ----
        def linear(x, w, out, extra_add=None):
            """out = x @ W^T + b (+ extra_add). x, out: [128, E] sbuf f32."""
            pst = psum.tile([E, E], F32, tag="pst")
            nc.tensor.transpose(pst[:], x[:], ident[:])
            xt = xpool.tile([E, E], F32, tag="xt")
            nc.any.tensor_copy(xt[:], pst[:])
            yps = psum.tile([128, E], F32, tag="yps")
            nc.tensor.matmul(yps[:], xt[:], wt[w][:], start=True, stop=True)
            if extra_add is None:
                nc.vector.tensor_add(out[:], yps[:], bfull[w][:])
            else:
                tmp = xpool.tile([128, E], F32, tag="lin_tmp")
                nc.vector.tensor_add(tmp[:], yps[:], bfull[w][:])
                nc.vector.tensor_add(out[:], tmp[:], extra_add[:])

        def layernorm(x, ln, out):
            stats = small.tile([128, 6], F32, tag="bn_stats")
            nc.vector.bn_stats(stats[:], x[:])
            mv = small.tile([128, 2], F32, tag="bn_mv")
            nc.vector.bn_aggr(mv[:], stats[:])
            std = small.tile([128, 1], F32, tag="std")
            nc.scalar.activation(std[:], mv[:, 1:2],
                                 mybir.ActivationFunctionType.Sqrt,
                                 bias=eps_t[:], scale=1.0)
            rstd = small.tile([128, 1], F32, tag="rstd")
            nc.vector.reciprocal(rstd[:], std[:])
            xn = xpool.tile([128, E], F32, tag="ln_xn")
            nc.vector.tensor_scalar(xn[:], x[:], mv[:, 0:1], rstd[:],
                                    mybir.AluOpType.subtract,
                                    mybir.AluOpType.mult)
            xg = xpool.tile([128, E], F32, tag="ln_xg")
            nc.vector.tensor_mul(xg[:], xn[:], lngf[ln][:])
            nc.vector.tensor_add(out[:], xg[:], lnbf[ln][:])

        def attention(q, kd, vd, s_tot, n_prev, kv_extra, masked, a_out):
            """Batched MHA decode, raw-reshape head semantics.
            q: [128, E] sbuf f32. kd/vd: dram [bl, n_prev, E] flat-viewed.
            s_tot: total positions per head (n_prev, or n_prev+1 w/ extra).
            kv_extra: None or (k_new, v_new) [128, E] f32 appended flat-end.
            a_out: [128, E] sbuf f32."""
            nflat_prev = n_prev * E
            ntiles = (s_tot + th - 1) // th
            nchunk = (s_tot + ch - 1) // ch
            kflat = kd[:].rearrange("b t e -> b (t e)")
            vflat = vd[:].rearrange("b t e -> b (t e)")
            dparts = acc.tile([128, H, ntiles], F32, tag="dparts")
            oparts = acc.tile([128, H, ntiles, HD], F32, tag="oparts")
            for h in range(H):
                # q for this head, pre-scaled by 1/sqrt(HD), in f16
                qh16 = small.tile([128, HD], F16, tag="qh16", name="qh16")
                nc.vector.tensor_scalar(qh16[:], q[:, h * HD:(h + 1) * HD],
                                        0.25, None, mybir.AluOpType.mult)
                qb = qh16[:].unsqueeze(1).broadcast_to([128, th, HD])
                for c in range(nchunk):
                    c0 = c * ch                      # chunk start position
                    cp = min(ch, s_tot - c0)         # positions in chunk
                    f0 = h * s_tot * HD + c0 * HD    # flat float offset
                    avail = max(0, min(cp * HD, nflat_prev - f0))
                    ndp = avail // HD                # positions from dram
                    kvdt = F16 if mode in ("full", "dmaonly") else F32
                    dma_eng = nc.sync if mode == "dmaonly_hw" else nc.gpsimd
                    kc = kpool.tile([128, ch, HD], kvdt, tag="kc")
                    vc = vpool.tile([128, ch, HD], kvdt, tag="vc")
                    if ndp > 0:
                        dma_eng.dma_start(
                            out=kc[:, :ndp, :],
                            in_=kflat[:, f0:f0 + ndp * HD].rearrange(
                                "b (t d) -> b t d", d=HD))
                        dma_eng.dma_start(
                            out=vc[:, :ndp, :],
                            in_=vflat[:, f0:f0 + ndp * HD].rearrange(
                                "b (t d) -> b t d", d=HD))
                    if ndp < cp:
                        # tail comes from the freshly-computed k/v token
                        e0 = f0 + ndp * HD - nflat_prev
                        ncp = (cp - ndp) * HD
                        nc.vector.tensor_copy(
                            kc[:, ndp:cp, :],
                            kv_extra[0][:, e0:e0 + ncp].rearrange(
                                "b (t d) -> b t d", d=HD))
                        nc.vector.tensor_copy(
                            vc[:, ndp:cp, :],
                            kv_extra[1][:, e0:e0 + ncp].rearrange(
                                "b (t d) -> b t d", d=HD))
                    ctiles = (cp + th - 1) // th
                    if cp < ctiles * th:
                        # zero-pad so padded scores exp to 0 via the NEG
                        # negmask pad (and pv pad is 0 * 0)
                        nc.vector.memset(kc[:, cp:ctiles * th, :], 0.0)
                        nc.vector.memset(vc[:, cp:ctiles * th, :], 0.0)
                    if mode.startswith("dmaonly"):
                        nc.vector.tensor_copy(
                            dparts[:, h, (c0 // th):(c0 // th) + 1],
                            kc[:, 0, 0:1])
                        nc.vector.tensor_copy(oparts[:, h, c0 // th, :],
                                              vc[:, 0, :])
                        continue
                    for i in range(ctiles):
                        t0 = i * th                  # offset within chunk
                        g0 = c0 + t0                 # global position
                        it = g0 // th                # global tile index
                        kt = kc[:, t0:t0 + th, :]
                        vt = vc[:, t0:t0 + th, :]
                        prod = ppool.tile([128, th, HD], F16, tag="prod")
                        nc.vector.tensor_mul(prod[:], kt, qb)
                        t8 = spool.tile([128, th, 8], F16, tag="t8")
                        nc.vector.tensor_add(t8[:], prod[:, :, 0:8],
                                             prod[:, :, 8:16])
                        t4 = spool.tile([128, th, 4], F16, tag="t4")
                        nc.vector.tensor_add(t4[:], t8[:, :, 0:4],
                                             t8[:, :, 4:8])
                        t2 = spool.tile([128, th, 2], F16, tag="t2")
                        nc.vector.tensor_add(t2[:], t4[:, :, 0:2],
                                             t4[:, :, 2:4])
                        s_raw = spool.tile([128, th], F16, tag="s_raw")
                        nc.vector.tensor_add(s_raw[:], t2[:, :, 0],
                                             t2[:, :, 1])
                        if masked:
                            s_in = spool.tile([128, th], F16, tag="s_msk")
                            nc.vector.tensor_add(s_in[:], s_raw[:],
                                                 negmask[:, g0:g0 + th])
                        else:
                            s_in = s_raw
                        # ACT: pbig[p,t,d] = exp(s[p,t]) replicated over d;
                        # accum_out = HD * sum_t exp (corrected at combine)
                        pbig = ppool.tile([128, th, HD], F16, tag="pbig")
                        nc.scalar.activation(
                            pbig[:],
                            s_in[:].unsqueeze(2).broadcast_to([128, th, HD]),
                            mybir.ActivationFunctionType.Exp,
                            accum_out=dparts[:, h, it:it + 1])
                        pv = ppool.tile([128, th, HD], F16, tag="pv")
                        nc.vector.tensor_mul(pv[:], vt, pbig[:])
                        pv2 = ppool.tile([128, th // 2, HD], F16, tag="pv2")
                        nc.vector.tensor_add(pv2[:], pv[:, 0:th // 2, :],
                                             pv[:, th // 2:th, :])
                        pv3 = ppool.tile([128, th // 4, HD], F16, tag="pv3")
                        nc.vector.tensor_add(pv3[:], pv2[:, 0:th // 4, :],
                                             pv2[:, th // 4:th // 2, :])
                        pv4 = ppool.tile([128, th // 8, HD], F16, tag="pv4")
                        nc.vector.tensor_add(pv4[:], pv3[:, 0:th // 8, :],
                                             pv3[:, th // 8:th // 4, :])
                        pv5 = ppool.tile([128, th // 16, HD], F16, tag="pv5")
                        nc.vector.tensor_add(pv5[:], pv4[:, 0:th // 16, :],
                                             pv4[:, th // 16:th // 8, :])
                        nc.vector.tensor_reduce(
                            oparts[:, h, it, :], pv5[:].transpose([0, 2, 1]),
                            mybir.AxisListType.X, mybir.AluOpType.add)
            d = small.tile([128, H], F32, tag="attn_d")
            nc.vector.tensor_reduce(d[:], dparts[:], mybir.AxisListType.X,
                                    mybir.AluOpType.add)
            r = small.tile([128, H], F32, tag="attn_r")
            nc.vector.reciprocal(r[:], d[:])
            o = xpool.tile([128, E], F32, tag="attn_o")
            nc.vector.tensor_reduce(o[:].rearrange("p (h d) -> p h d", h=H),
                                    oparts[:].transpose([0, 1, 3, 2]),
                                    mybir.AxisListType.X, mybir.AluOpType.add)
            rb = r[:].unsqueeze(2).broadcast_to([128, H, HD])
            # a = o * (HD / d): the HD factor undoes the broadcast accum
            nc.vector.scalar_tensor_tensor(
                a_out[:].rearrange("p (h d) -> p h d", h=H),
                o[:].rearrange("p (h d) -> p h d", h=H), float(HD), rb,
                mybir.AluOpType.mult, mybir.AluOpType.mult)

        # ---- model ----
        for _rep in range(repeat):
            k_sa = xpool.tile([128, E], F32, tag="k_sa", name="k_sa")
            linear(ht, "Wk", k_sa)
            v_sa = xpool.tile([128, E], F32, tag="v_sa", name="v_sa")
            linear(ht, "Wv", v_sa)

            a_sa = xpool.tile([128, E], F32, tag="a_sa", name="a_sa")
            attention(ht, d_ksa, d_vsa, t_prev + 1, t_prev, (k_sa, v_sa),
                      False, a_sa)

            h1 = xpool.tile([128, E], F32, tag="h1", name="h1")
            linear(a_sa, "W0sa", h1, extra_add=ht)
            h1ln = xpool.tile([128, E], F32, tag="h1ln", name="h1ln")
            layernorm(h1, "ln_sa", h1ln)

            q = xpool.tile([128, E], F32, tag="q", name="q")
            linear(h1ln, "Wqatt", q)
            a_att = xpool.tile([128, E], F32, tag="a_att", name="a_att")
            attention(q, d_katt, d_vatt, n1, n1, None, True, a_att)

            h2 = xpool.tile([128, E], F32, tag="h2", name="h2")
            linear(a_att, "W0att", h2, extra_add=h1ln)
            h2ln = xpool.tile([128, E], F32, tag="h2ln", name="h2ln")
            layernorm(h2, "ln_sa", h2ln)

            ff_pre = xpool.tile([128, E], F32, tag="ff_pre", name="ff_pre")
            linear(h2ln, "W1", ff_pre)
            ff = xpool.tile([128, E], F32, tag="ff", name="ff")
            nc.scalar.activation(ff[:], ff_pre[:],
                                 mybir.ActivationFunctionType.Relu)
            h3 = xpool.tile([128, E], F32, tag="h3", name="h3")
            linear(ff, "W2", h3, extra_add=h2ln)
            h3ln = xpool.tile([128, E], F32, tag="h3ln", name="h3ln")
            layernorm(h3, "ln_ff", h3ln)

            nc.sync.dma_start(out=d_out[:], in_=h3ln[:])

    nc.compile()
    return nc


_NC_CACHE = {}


def _get_nc():
    key = (BL, N1, T_PREV)
    if key not in _NC_CACHE:
        _NC_CACHE[key] = build_kernel()
    return _NC_CACHE[key]


def make_in_maps(inputs, bl=BL, ncores=NCORES):
    """Shard batch dim across cores; replicate weights."""
    in_maps = []
    for c in range(ncores):
        sl = slice(c * bl, (c + 1) * bl)
        m = {}
        # asarray(dtype=...) only copies when conversion is needed; batch
        # slices of C-contiguous inputs pass through zero-copy.
        m["h_t"] = np.ascontiguousarray(np.asarray(
            inputs["h_t"], dtype=np.float32)[sl].reshape(bl, E))
        for k in ["K_att", "V_att", "K_sa_prev", "V_sa_prev"]:
            m[k] = np.ascontiguousarray(np.asarray(inputs[k],
                                                   dtype=np.float32)[sl])
        m["mask"] = np.ascontiguousarray(
            np.asarray(inputs["mask"], dtype=np.uint8)[sl])
        for w in WNAMES:
            m[w + "_w"] = np.ascontiguousarray(inputs[w + "_w"].astype(np.float32))
            m[w + "_b"] = np.ascontiguousarray(
                inputs[w + "_b"].reshape(1, E).astype(np.float32))
        for ln in ["ln_sa", "ln_ff"]:
            m[ln + "_g"] = np.ascontiguousarray(
                inputs[ln + "_g"].reshape(1, E).astype(np.float32))
            m[ln + "_b"] = np.ascontiguousarray(
                inputs[ln + "_b"].reshape(1, E).astype(np.float32))
        in_maps.append(m)
    return in_maps


def kernel(**inputs):
    nc = _get_nc()
    in_maps = make_in_maps(inputs)
    res = run_bass_kernel_spmd(nc, in_maps, core_ids=list(range(NCORES)))
    outs = [res.results[i]["out"].reshape(BL, 1, E) for i in range(NCORES)]
    return np.concatenate(outs, axis=0)
